# revision 53
# baseline (speedup 1.0000x reference)
"""GCN graph encoder on 8 Trainium2 NeuronCores (Bass/Tile SPMD).

Design: nodes (and incident edges, by dst) split 8 ways. Per GCN layer each
core computes raw projections (h @ W)^T with the PE in a transposed layout
[C, nodes], transposes per 128-node window while scaling by dinv_src, and
writes a bf16 message table slice that one AllGather per layer
replicates (12.8 MB); phase A of layer l+1 is interleaved into layer l's
phase-B window loop (per-group ag_in DMA slices) so the AllGather input
is ready the moment the last epilogue lands.
Aggregation: per-edge message rows are fetched with dma_gather as 256B
bf16 row-PAIRS (row = 2*NQ*u + 2q + rp) on FOUR SWDGE queues — the
aggregate descriptor rate saturates around ~3 ns/desc at 4 queues, the
16-DMA-engine small-descriptor wall, so descriptor COUNT is the cost.
Three count reducers: (1) per-core node permutation deals nodes to
windows snake-wise by in-degree so window in-degree sums are near-equal;
(2) each message is written TWICE into the table (copy offset CPO odd,
flipping row parity) and every edge greedily picks whichever of its two
(queue,parity) colors balances its (core,window) cell — flattening the
max-over-cores group counts that set the padding; (3) edges pack into
per-(queue,parity) streams with cross-core-uniform cumulative window
boundaries (no per-window tile rounding); tiles may straddle windows —
each (tile, window) pair gets its own one-hot column, foreign slots
masked to -1. Net: ~201.7k descriptors/core/layer vs the 200.2k floor.
Each tile is reduced on the PE by a bf16 one-hot matmul (one-hot built
on DVE from precomputed dst codes), accumulating all tiles of a window
plus an identity-matmul self-loop term in PSUM. Mean-pool partials
reduce across cores with one AllReduce; the output MLP is replicated.
"""
import sys, os
sys.path.insert(0, "/opt/trn_rl_repo")
import numpy as np
import ml_dtypes

SKIP_AG = os.environ.get("SKIP_AG") == "1"
SKIP_GATH = os.environ.get("SKIP_GATH") == "1"
SKIP_OH = os.environ.get("SKIP_OH") == "1"

N = 100000
E = 1600000
F = 128
C = 64
NH = 256
NOUT = 128
L = 4            # hidden layers (total GCN layers = L + 1)
G = 512
W = 8            # cores
NL = N // W      # 12500 local nodes
P = 128
NWIN = (NL + P - 1) // P            # 98 windows (last partial: 84 nodes)
NLP = NWIN * P                      # 12544 padded local nodes
CPO = 12503                         # second-copy row offset (odd: parity flips)
BLK = 2 * CPO                       # 25006 rows per core block (2 copies)
TROWS = W * BLK                     # 200048 table rows
NQ = 4                              # SWDGE queues (elem q = (row//EROWS) % NQ)
EROWS = 2                           # table rows per gather element
CT = 12                             # tiles per gather call
GB = 4                              # gather ring depth
SP = False                          # dma_gather single_packet (True wedges HW)

_cache = {}


def _build_host_structures(edge_index, batch):
    src = edge_index[0].astype(np.int64)
    dst = edge_index[1].astype(np.int64)
    deg = np.bincount(dst, minlength=N).astype(np.float64) + 1.0
    dinv = (1.0 / np.sqrt(deg)).astype(np.float32)          # [N]
    counts_g = np.bincount(batch, minlength=G).astype(np.float32)

    # Per-core node permutation: deal nodes to windows snake-wise by
    # descending in-degree so window in-degree sums are near-equal.
    pms = []                             # pms[c][pos] = local node id
    posn = np.empty((W, NL), np.int64)   # posn[c][node] = position
    caps_full = [P] * (NWIN - 1) + [NL - (NWIN - 1) * P]
    for c in range(W):
        degl = np.bincount(dst[(dst >= c * NL) & (dst < (c + 1) * NL)]
                           - c * NL, minlength=NL)
        order = np.argsort(-degl, kind="stable")
        caps = list(caps_full)
        fill = [[] for _ in range(NWIN)]
        wseq = list(range(NWIN)) + list(range(NWIN - 1, -1, -1))
        i = 0
        for nd in order:
            while True:
                wsel = wseq[i % len(wseq)]
                i += 1
                if caps[wsel]:
                    break
            caps[wsel] -= 1
            fill[wsel].append(nd)
        pm = np.concatenate([np.array(f, np.int64) for f in fill])
        pms.append(pm)
        posn[c][pm] = np.arange(NL)

    # two table copies per src (copy offset CPO is odd, so the second
    # copy flips row parity): each edge picks one of two (q,rp) colors.
    c_src = src // NL
    p_src = posn[c_src, src % NL]
    rowA = BLK * c_src + p_src
    rowB = rowA + CPO
    colA = ((rowA // EROWS) % NQ * EROWS + rowA % EROWS).astype(np.int64)
    colB = ((rowB // EROWS) % NQ * EROWS + rowB % EROWS).astype(np.int64)

    c_dst = dst // NL
    ed_all = posn[c_dst, dst % NL]
    w_all = ed_all // P
    cell = (c_dst * NWIN + w_all).astype(np.int64)

    # greedy 2-choice per (core, window) cell: balance the 8 colors
    corder = np.argsort(cell, kind="stable")
    cA = colA[corder].tolist()
    cB = colB[corder].tolist()
    cc = cell[corder].tolist()
    pk = [False] * E
    cnt_row = [0] * (NQ * EROWS)
    prev = -1
    for i in range(E):
        ci = cc[i]
        if ci != prev:
            cnt_row = [0] * (NQ * EROWS)
            prev = ci
        a = cA[i]
        b = cB[i]
        if cnt_row[b] < cnt_row[a]:
            pk[i] = True
            cnt_row[b] += 1
        else:
            cnt_row[a] += 1
    pick = np.zeros(E, np.bool_)         # True -> copy B
    pick[corder] = np.array(pk, np.bool_)
    row = np.where(pick, rowB, rowA)
    u_all = row // (EROWS * NQ)
    q_all = ((row // EROWS) % NQ).astype(np.int64)
    rp_all = (row % EROWS).astype(np.int64)

    # group sizes per core
    sizes = np.zeros((W, NWIN, NQ, EROWS), np.int64)
    per_core = []
    for c in range(W):
        m = c_dst == c
        ed = ed_all[m]
        w_ = ed // P
        dr = ed % P
        q2 = q_all[m]
        rp = rp_all[m]
        u = u_all[m]
        np.add.at(sizes[c], (w_, q2, rp), 1)
        per_core.append((w_, q2, rp, u, dr))

    # packed streams: per (q,rp) stream, window w's segment spans slots
    # [B[w-1], B[w]) where increments are the max-over-cores group count
    # (cross-core uniform; tiles straddle windows, oh masks foreign slots)
    mx = sizes.max(axis=0)                                  # [NWIN,NQ,2]
    B = np.cumsum(mx, axis=0)                               # segment ends
    B0 = B - mx                                             # segment starts
    NTS = int(-(-B[-1].max() // P))                         # tiles per stream
    # rel columns in program order: (w, q, rp, tile)
    colstart = np.zeros((NWIN, NQ, EROWS), np.int64)
    col = 0
    for w_ in range(NWIN):
        for q2 in range(NQ):
            for rp in range(EROWS):
                colstart[w_, q2, rp] = col
                if B[w_, q2, rp] > B0[w_, q2, rp]:
                    col += ((B[w_, q2, rp] - 1) // P
                            - B0[w_, q2, rp] // P + 1)
    NT = col

    def wrap16(a):
        n = len(a)
        return np.tile(a.reshape(n // 16, 16).T, (8, 1))

    inputs_per_core = []
    for c in range(W):
        w_, q2, rp, u, dr = per_core[c]
        # slot rank within (q,rp,w) group, ordered by u
        key = ((q2 * EROWS + rp) * NWIN + w_)
        order = np.lexsort((u, key))
        ks_key, ks_u, ks_w, ks_q2, ks_rp, ks_dr = (
            key[order], u[order], w_[order], q2[order], rp[order], dr[order])
        uniq, first_idx = np.unique(ks_key, return_index=True)
        grp_start = np.zeros(len(ks_key), np.int64)
        grp_start[first_idx] = first_idx
        grp_start = np.maximum.accumulate(grp_start)
        rank = np.arange(len(ks_key)) - grp_start
        assert (rank < mx[ks_w, ks_q2, ks_rp]).all()
        pos = B0[ks_w, ks_q2, ks_rp] + rank                 # slot in stream

        # idx streams: queue q's EROWS substream tiles interleaved
        # (qt = EROWS*t + rp). Padding slots read u=0 — the one-hot's
        # rel=-1 zeroes them, and all table values are finite.
        qstreams = []
        for q2v in range(NQ):
            arr = np.zeros((EROWS * NTS, P), np.int16)
            for rpv in range(EROWS):
                sel = (ks_q2 == q2v) & (ks_rp == rpv)
                p_ = pos[sel]
                arr[EROWS * (p_ // P) + rpv, p_ % P] = \
                    ks_u[sel].astype(np.int16)
            qstreams.append(arr.reshape(-1))
        gidx = np.concatenate([wrap16(s) for s in qstreams], axis=1)

        # rel codes: column = colstart[w,q,rp] + (tile - first tile of segment)
        rel = np.full((P, NT), -1.0, np.float32)
        cole = (colstart[ks_w, ks_q2, ks_rp]
                + pos // P - B0[ks_w, ks_q2, ks_rp] // P)
        rel[pos % P, cole] = ks_dr.astype(np.float32)
        rel = rel.astype(ml_dtypes.bfloat16)

        pm = pms[c]
        dv = dinv[c * NL + pm]
        dflat = np.ones(NLP, np.float32)
        dflat[:NL] = dv
        dinvcol = dflat.reshape(NWIN, P).T                  # [p, w]
        dinvrep = np.tile(dflat[None, :], (C, 1)).astype(ml_dtypes.bfloat16)

        bl = batch[c * NL + pm].astype(np.int64)
        base_g = int(batch[c * NL])
        brel = np.full(NLP, -1.0, np.float32)
        brel[:NL] = (bl - base_g).astype(np.float32)
        assert 0 <= brel[:NL].min() and brel.max() < 128, "graph span"
        batch_rel = brel.reshape(NWIN, P).T                 # [p, w]
        pool_off = (base_g + np.arange(P)).astype(np.int32)[:, None]

        inputs_per_core.append(dict(
            gidx=gidx,
            rel=rel,
            dinvcol=dinvcol.astype(np.float32),
            dinvrep=dinvrep,
            brel=batch_rel.astype(np.float32),
            poff=pool_off,
            pm=pm,
        ))

    shared = dict(B=B, B0=B0, NTS=NTS, colstart=colstart, NT=NT,
                  counts_g=counts_g)
    return inputs_per_core, shared


def _build_program(shared, repeat=None):
    import concourse.bass as bass
    import concourse.bacc as bacc
    import concourse.mybir as mybir
    import concourse.tile as tile
    from concourse.masks import make_identity

    REPEAT = int(os.environ.get("REPEAT", "1")) if repeat is None else repeat
    B = shared["B"]
    B0 = shared["B0"]
    NTS = shared["NTS"]
    NT = shared["NT"]
    GW = EROWS * NTS * 8                 # gidx cols per queue

    fp32 = mybir.dt.float32
    bf16 = mybir.dt.bfloat16
    nc = bacc.Bacc("TRN2", target_bir_lowering=False, debug=False,
                   num_devices=W, num_swdge_queues=NQ)

    xt_in = nc.dram_tensor("xt", [F, NLP], fp32, kind="ExternalInput")
    W0_in = nc.dram_tensor("w0", [F, C], fp32, kind="ExternalInput")
    Wh_in = nc.dram_tensor("wh", [C, L * C], fp32, kind="ExternalInput")
    BbT_in = nc.dram_tensor("bbt", [C, L + 1], fp32, kind="ExternalInput")
    dcol_in = nc.dram_tensor("dinvcol", [P, NWIN], fp32, kind="ExternalInput")
    drep_in = nc.dram_tensor("dinvrep", [C, NLP], bf16, kind="ExternalInput")
    gidx_in = nc.dram_tensor("gidx", [P, NQ * GW], mybir.dt.int16,
                             kind="ExternalInput")
    rel_in = nc.dram_tensor("rel", [P, NT], bf16, kind="ExternalInput")
    iob_in = nc.dram_tensor("iob", [P, P], bf16, kind="ExternalInput")
    idb_in = nc.dram_tensor("idb", [P, P], bf16, kind="ExternalInput")
    brel_in = nc.dram_tensor("brel", [P, NWIN], fp32, kind="ExternalInput")
    poff_in = nc.dram_tensor("poff", [P, 1], mybir.dt.int32, kind="ExternalInput")
    cnts_in = nc.dram_tensor("cnts", [P, G // P], fp32, kind="ExternalInput")
    iota_in = nc.dram_tensor("iota", [P, P], fp32, kind="ExternalInput")
    W1_in = nc.dram_tensor("w1", [C, NH], fp32, kind="ExternalInput")
    b1_in = nc.dram_tensor("b1", [P, 2], fp32, kind="ExternalInput")
    W2_in = nc.dram_tensor("w2", [P, 2 * NOUT], fp32, kind="ExternalInput")
    b2_in = nc.dram_tensor("b2", [NOUT, 1], fp32, kind="ExternalInput")
    out_t = nc.dram_tensor("out_t", [NOUT, G], fp32, kind="ExternalOutput")

    with tile.TileContext(nc, num_cores=W) as tc:
        with (
            tc.tile_pool(name="const", bufs=1) as constp,
            tc.tile_pool(name="state", bufs=1) as statep,
            tc.tile_pool(name="xtp", bufs=2) as xtp,
            tc.tile_pool(name="work", bufs=3) as workp,
            tc.tile_pool(name="gat", bufs=GB) as gatp,
            tc.tile_pool(name="ohp", bufs=4) as ohp,
            tc.tile_pool(name="ps", bufs=2, space="PSUM") as psp,
            tc.tile_pool(name="psw", bufs=2, space="PSUM") as pswp,
            tc.tile_pool(name="dram", bufs=1, space="DRAM") as dramp,
        ):
            # ---------- constants ----------
            gidx = constp.tile([P, NQ * GW], mybir.dt.int16)
            nc.sync.dma_start(out=gidx[:], in_=gidx_in[:])
            rel = constp.tile([P, NT], bf16)
            nc.sync.dma_start(out=rel[:], in_=rel_in[:])
            iob = constp.tile([P, P], bf16)
            nc.sync.dma_start(out=iob[:], in_=iob_in[:])
            idb = constp.tile([P, P], bf16)
            nc.sync.dma_start(out=idb[:], in_=idb_in[:])
            dinvcol = constp.tile([P, NWIN], fp32)
            nc.sync.dma_start(out=dinvcol[:], in_=dcol_in[:])
            dinvrep = constp.tile([C, NLP], bf16)
            nc.sync.dma_start(out=dinvrep[:], in_=drep_in[:])
            bbT = constp.tile([C, L + 1], fp32)
            nc.sync.dma_start(out=bbT[:], in_=BbT_in[:])
            w0 = constp.tile([F, C], fp32)
            nc.sync.dma_start(out=w0[:], in_=W0_in[:])
            wh = constp.tile([C, L * C], fp32)
            nc.sync.dma_start(out=wh[:], in_=Wh_in[:])
            ident = constp.tile([P, P], fp32)
            make_identity(nc, ident[:])

            hT = statep.tile([C, NLP], fp32, name="hT")
            nc.vector.memset(hT[:, NL:NLP], 0.0)
            m_sb = statep.tile([P, NWIN, C], bf16, name="m_sb")
            h_w = m_sb

            tables = [dramp.tile([TROWS, C], bf16, addr_space="Shared",
                                 name=f"table{li}")
                      for li in range(REPEAT * (L + 1))]
            ag_in = dramp.tile([BLK, C], bf16)
            ztile = constp.tile([2, C], bf16)
            nc.vector.memset(ztile[:], 0.0)
            nc.sync.dma_start(out=ag_in[NL:NL + 2, :], in_=ztile[:])

            # ---------- layers ----------
            NG = NLP // 512 + (1 if NLP % 512 else 0)       # 25 col groups
            NLAY = REPEAT * (L + 1)

            def phase_a_group(lay, g):
                """Phase A for col group g of layer lay + its ag_in slice."""
                layer = lay % (L + 1)
                lhs = w0[:] if layer == 0 else wh[:, (layer - 1) * C:layer * C]
                c0 = g * 512
                ncol = min(512, NLP - c0)
                if layer == 0:
                    rhs_t = xtp.tile([F, 512], fp32, tag="xt")
                    nc.sync.dma_start(out=rhs_t[:, :ncol],
                                      in_=xt_in[:, c0:c0 + ncol])
                    rhs = rhs_t[:, :ncol]
                else:
                    rhs = hT[:, c0:c0 + ncol]
                psA = psp.tile([C, 512], fp32, tag="psA")
                nc.tensor.matmul(psA[:, :ncol], lhs, rhs,
                                 start=True, stop=True)
                wk = workp.tile([C, 512], fp32, tag="wk")
                nc.scalar.copy(out=wk[:, :ncol], in_=psA[:, :ncol])
                w_hi = 0
                for j in range(ncol // P):
                    w_ = (c0 + j * P) // P
                    pt = psp.tile([P, C], fp32, tag="pt")
                    nc.tensor.transpose(
                        out=pt[:], in_=wk[:, j * P:(j + 1) * P],
                        identity=ident[:C, :C])
                    nc.vector.tensor_scalar_mul(
                        m_sb[:, w_, :], pt[:], dinvcol[:, w_:w_ + 1])
                    w_hi = w_
                wlo = c0 // P
                whi = min(w_hi + 1, NWIN - 1)
                for off in (0, CPO):
                    if wlo < whi:
                        nc.sync.dma_start(
                            out=ag_in[off + wlo * P:off + whi * P, :]
                                .rearrange("(w p) c -> p w c", p=P),
                            in_=m_sb[:, wlo:whi, :])
                    if w_hi == NWIN - 1:
                        nc.sync.dma_start(
                            out=ag_in[off + (NWIN - 1) * P:off + NL, :],
                            in_=m_sb[0:NL - (NWIN - 1) * P, NWIN - 1, :])

            def emit_ag(lay):
                if not SKIP_AG:
                    nc.gpsimd.collective_compute(
                        "AllGather", mybir.AluOpType.bypass,
                        replica_groups=[list(range(W))],
                        ins=[ag_in.opt()],
                        outs=[tables[lay].opt()],
                    )

            for g in range(NG):
                phase_a_group(0, g)
            emit_ag(0)

            for rep in range(REPEAT):
              for layer in range(L + 1):
                lay = rep * (L + 1) + layer
                tbl = tables[lay]
                chunks = tbl[:].rearrange("(u q2 two) c -> q2 u (two c)",
                                          q2=NQ, two=EROWS)

                # Phase B: gather streams + one-hot matmul aggregation.
                # Queue q's tile stream interleaves (rp0,rp1) as qt = 2t+rp.
                col0_16 = [q * GW for q in range(NQ)]
                calls = [[] for _ in range(NQ)]   # (gt, lo, hi) per q
                hi_issued = [0] * NQ

                def ensure(q2, qt):
                    while qt >= hi_issued[q2]:
                        lo = hi_issued[q2]
                        hi = min(EROWS * NTS, lo + CT)
                        n = (hi - lo) * P
                        gt = gatp.tile([P, CT, EROWS * C], bf16,
                                       tag=f"g{q2}")
                        if not SKIP_GATH:
                            nc.gpsimd.dma_gather(
                                gt[:, :hi - lo, :], chunks[q2],
                                gidx[:, col0_16[q2] + lo * 8:
                                     col0_16[q2] + hi * 8],
                                n, n, EROWS * C, elem_step=NQ * EROWS * C,
                                single_packet=SP, queue_num=q2)
                        calls[q2].append((gt, lo, hi))
                        hi_issued[q2] = hi

                def gt_at(q2, qt):
                    ensure(q2, qt)
                    for gt, lo, hi in reversed(calls[q2][-3:]):
                        if lo <= qt < hi:
                            return gt, qt - lo
                    raise AssertionError("non-monotone gather consumption")

                colctr = 0
                for w_ in range(NWIN):
                    ps_w = pswp.tile([C, P], fp32, tag="psw")
                    first = True
                    for q2 in range(NQ):
                        for rp in range(EROWS):
                            pstart = int(B0[w_, q2, rp])
                            pend = int(B[w_, q2, rp])
                            if pend <= pstart:
                                continue
                            for t in range(pstart // P,
                                           (pend - 1) // P + 1):
                                cc = colctr
                                colctr += 1
                                if SKIP_GATH:
                                    continue
                                gt, loc = gt_at(q2, EROWS * t + rp)
                                oh = ohp.tile([P, P], bf16, tag="oh")
                                nc.vector.tensor_tensor(
                                    out=oh[:],
                                    in0=rel[:, cc:cc + 1]
                                        .to_broadcast([P, P]),
                                    in1=iob[:],
                                    op=mybir.AluOpType.is_equal)
                                nc.tensor.matmul(
                                    ps_w[:], gt[:, loc, rp * C:(rp + 1) * C],
                                    oh[:], start=first, stop=False)
                                first = False
                    # self-loop: += m_w^T via identity matmul
                    nc.tensor.matmul(ps_w[:], m_sb[:, w_, :], idb[:],
                                     start=first, stop=True)
                    if w_ == NWIN - 1:
                        assert colctr == NT, (colctr, NT)
                    # epilogue: hT_w = relu(dinv * ps_w + b)
                    tmp = workp.tile([C, P], fp32, tag="ep")
                    nc.vector.tensor_tensor(
                        out=tmp[:], in0=ps_w[:],
                        in1=dinvrep[:, w_ * P:(w_ + 1) * P],
                        op=mybir.AluOpType.mult)
                    nc.scalar.activation(
                        hT[:, w_ * P:(w_ + 1) * P], tmp[:],
                        mybir.ActivationFunctionType.Relu,
                        bias=bbT[:, layer:layer + 1])
                    if lay + 1 < NLAY and w_ % 4 == 3:
                        phase_a_group(lay + 1, w_ // 4)
                if lay + 1 < NLAY:
                    phase_a_group(lay + 1, NG - 1)
                    emit_ag(lay + 1)

            # ---------- final transposes for pooling ----------
            for w_ in range(NWIN):
                pt = psp.tile([P, C], fp32, tag="pt")
                nc.tensor.transpose(
                    out=pt[:], in_=hT[:, w_ * P:(w_ + 1) * P],
                    identity=ident[:C, :C])
                nc.any.tensor_copy(out=h_w[:, w_, :], in_=pt[:])

            # ---------- pooling ----------
            brel = constp.tile([P, NWIN], fp32)
            nc.sync.dma_start(out=brel[:], in_=brel_in[:])
            iota = constp.tile([P, P], fp32)
            nc.sync.dma_start(out=iota[:], in_=iota_in[:])
            pool_ps = pswp.tile([C, P], fp32, tag="psw")
            for w_ in range(NWIN):
                ohg = ohp.tile([P, P], bf16, tag="oh")
                nc.vector.tensor_tensor(
                    out=ohg[:],
                    in0=brel[:, w_:w_ + 1].to_broadcast([P, P]),
                    in1=iota[:], op=mybir.AluOpType.is_equal)
                nc.tensor.matmul(pool_ps[:], h_w[:, w_, :], ohg[:],
                                 start=(w_ == 0), stop=(w_ == NWIN - 1))
            poolT = constp.tile([C, P], fp32)
            nc.any.tensor_copy(out=poolT[:], in_=pool_ps[:])
            pool_n = constp.tile([P, C], fp32)
            pp = psp.tile([P, C], fp32, tag="pt")
            nc.tensor.transpose(out=pp[:], in_=poolT[:], identity=ident[:C, :C])
            nc.any.tensor_copy(out=pool_n[:], in_=pp[:])

            ar_in = dramp.tile([640, C], fp32)
            ar_out = dramp.tile([640, C], fp32, addr_space="Shared")
            zt = constp.tile([P, C], fp32)
            nc.vector.memset(zt[:], 0.0)
            for z5 in range(5):
                nc.sync.dma_start(out=ar_in[z5 * P:(z5 + 1) * P, :], in_=zt[:])
            poff = constp.tile([P, 1], mybir.dt.int32)
            nc.sync.dma_start(out=poff[:], in_=poff_in[:])
            nc.gpsimd.indirect_dma_start(
                out=ar_in[:],
                out_offset=bass.IndirectOffsetOnAxis(ap=poff[:, :1], axis=0),
                in_=pool_n[:], in_offset=None)
            nc.gpsimd.collective_compute(
                "AllReduce", mybir.AluOpType.add,
                replica_groups=[list(range(W))],
                ins=[ar_in.opt()], outs=[ar_out.opt()],
            )
            pools = constp.tile([P, G // P, C], fp32)
            nc.sync.dma_start(
                out=pools[:],
                in_=ar_out[0:G, :].rearrange("(w p) c -> p w c", p=P))
            cnts = constp.tile([P, G // P], fp32)
            nc.sync.dma_start(out=cnts[:], in_=cnts_in[:])
            cmax = constp.tile([P, G // P], fp32)
            nc.vector.tensor_scalar_max(cmax[:], cnts[:], 1.0)
            crec = constp.tile([P, G // P], fp32)
            nc.vector.reciprocal(crec[:], cmax[:])
            for j in range(G // P):
                nc.vector.tensor_scalar_mul(
                    pools[:, j, :], pools[:, j, :], crec[:, j:j + 1])
            pT_ps = pswp.tile([C, G], fp32, tag="pT", bufs=1)
            for j in range(G // P):
                nc.tensor.transpose(out=pT_ps[:, j * P:(j + 1) * P],
                                    in_=pools[:, j, :], identity=ident[:])
            pT = constp.tile([C, G], fp32)
            nc.any.tensor_copy(out=pT[:], in_=pT_ps[:])
            # MLP
            w1t = constp.tile([C, NH], fp32)
            nc.sync.dma_start(out=w1t[:], in_=W1_in[:])
            b1t = constp.tile([P, 2], fp32)
            nc.sync.dma_start(out=b1t[:], in_=b1_in[:])
            w2t = constp.tile([P, 2 * NOUT], fp32)
            nc.sync.dma_start(out=w2t[:], in_=W2_in[:])
            b2t = constp.tile([NOUT, 1], fp32)
            nc.sync.dma_start(out=b2t[:], in_=b2_in[:])

            a1 = constp.tile([P, 2 * G], fp32)
            for half in range(2):
                z1 = pswp.tile([P, G], fp32, tag="z1", bufs=1)
                nc.tensor.matmul(z1[:], w1t[:, half * P:(half + 1) * P], pT[:],
                                 start=True, stop=True)
                nc.scalar.activation(
                    a1[:, half * G:(half + 1) * G], z1[:],
                    mybir.ActivationFunctionType.Relu,
                    bias=b1t[:, half:half + 1])
            z2 = pswp.tile([NOUT, G], fp32, tag="z1", bufs=1)
            nc.tensor.matmul(z2[:], w2t[:, 0:NOUT], a1[:, 0:G],
                             start=True, stop=False)
            nc.tensor.matmul(z2[:], w2t[:, NOUT:2 * NOUT], a1[:, G:2 * G],
                             start=False, stop=True)
            outs = constp.tile([NOUT, G], fp32)
            nc.vector.tensor_scalar_add(outs[:], z2[:], b2t[:, 0:1])
            nc.sync.dma_start(out=out_t[:], in_=outs[:])

    nc.finalize()
    return nc


def kernel(x, edge_index, batch, W0, b0, Wh, bh, W1, b1, W2, b2):
    x = np.asarray(x)
    edge_index = np.asarray(edge_index)
    batch = np.asarray(batch)

    assert x.shape == (N, F) and edge_index.shape == (2, E) and batch.shape == (N,)
    key = (int(edge_index[:, ::4097].astype(np.int64).sum()),
           int(np.asarray(batch[::997]).astype(np.int64).sum()))
    if key not in _cache:
        _cache.clear()
        inputs_per_core, shared = _build_host_structures(edge_index, batch)
        nc = _build_program(shared)
        _cache[key] = (nc, inputs_per_core, shared)
    nc, inputs_per_core, shared = _cache[key]

    bvec = np.concatenate([np.asarray(b0, np.float32)[None, :],
                           np.asarray(bh, np.float32)], axis=0)   # [L+1, C]
    bbT = np.ascontiguousarray(bvec.T)                            # [C, L+1]
    iota = np.tile(np.arange(P, dtype=np.float32)[None, :], (P, 1))
    iob = iota.astype(ml_dtypes.bfloat16)
    idb = np.eye(P, dtype=np.float32).astype(ml_dtypes.bfloat16)
    cnts = shared["counts_g"].reshape(G // P, P).T.astype(np.float32)

    xf = np.asarray(x, np.float32)
    in_maps = []
    for c in range(W):
        pc = inputs_per_core[c]
        xl = np.zeros((F, NLP), np.float32)
        xl[:, :NL] = xf[c * NL + pc["pm"]].T
        in_maps.append(dict(
            xt=np.ascontiguousarray(xl),
            w0=np.asarray(W0, np.float32),
            wh=np.ascontiguousarray(np.asarray(Wh, np.float32)
                                    .transpose(1, 0, 2).reshape(C, L * C)),
            bbt=bbT,
            dinvcol=pc["dinvcol"],
            dinvrep=pc["dinvrep"],
            gidx=pc["gidx"],
            rel=pc["rel"],
            iob=iob,
            idb=idb,
            brel=pc["brel"],
            poff=pc["poff"],
            cnts=cnts,
            iota=iota,
            w1=np.asarray(W1, np.float32),
            b1=np.ascontiguousarray(np.asarray(b1, np.float32)
                                    .reshape(2, P).T),
            w2=np.ascontiguousarray(np.asarray(W2, np.float32)
                                    .reshape(2, P, NOUT).transpose(1, 0, 2)
                                    .reshape(P, 2 * NOUT)),
            b2=np.asarray(b2, np.float32)[:, None],
        ))

    from concourse.bass_utils import run_bass_kernel_spmd
    _cache["last_run"] = (nc, in_maps)
    _cache["shared"] = shared
    res = run_bass_kernel_spmd(nc, in_maps, list(range(W)))
    out_t = res.results[0]["out_t"]          # [NOUT, G]
    return np.ascontiguousarray(out_t.T.astype(np.float32))


if __name__ == "__main__":
    pass

